# revision 1
# baseline (speedup 1.0000x reference)
"""Multi-Head Latent Attention (MLA) Trainium2 kernel, 8-core head-sharded.

Layout strategy: all device-side matmuls run with the contraction dim on
partitions ("transposed world"), with x and every weight pre-transposed on
the host. All big matmuls use float32r (TF32-like, ~1.5e-4 rel err, full
PE rate with 512-wide moving operands). Heads are sharded 2-per-core; each
core emits a partial out.T (its heads' contribution to the output
projection), summed and transposed on the host.

Pipeline order per query block qb: stage2(qb) -> attention(qb) [h1's
broadcast matmul deferred] -> stage1(qb+1) -> stage5(qb), so the softmax
reciprocal chain and next-block DMAs hide under stage-1 matmuls.
"""
import sys

sys.path.insert(0, "/opt/trn_rl_repo")

import numpy as np

import concourse.bass as bass
import concourse.tile as tile
from concourse import bacc, mybir
from concourse.bass_utils import run_bass_kernel_spmd

F32 = mybir.dt.float32
F32R = mybir.dt.float32r
AF = mybir.ActivationFunctionType
OP = mybir.AluOpType

N_CORES = 8
S = 2048          # sequence length
DM = 2048         # d_model
DL = 512          # d_latent
H = 16            # total heads
HC = H // N_CORES  # heads per core (2)
DH = 128          # head dim (content)
DHR = 64          # head dim (rope)
QB = 512          # query block
NQB = S // QB     # 4
KPB = QB // 128   # key chunks per query block (4)
NMC = DM // 128   # 16 model chunks
NLC = DL // 128   # 4 latent chunks
NKC = S // 128    # 16 key chunks
THETA = 10000.0

SCALE = float(1.0 / np.sqrt(np.float32(DH + DHR)))
E_HI = float(np.exp(np.float64(80.0) * SCALE))
E_LO = float(np.exp(np.float64(-80.0) * SCALE))

# Set by test.py to profile; harness path leaves these untouched.
TRACE = False
TRACE_KWARGS = {}
LAST_EXEC_TIME_NS = None
LAST_RESULTS = None

_CACHE = {}
MM_LABELS = {}


def _lbl(inst, label):
    try:
        MM_LABELS[inst.ins.name] = label
    except Exception:
        try:
            MM_LABELS[inst.name] = label
        except Exception:
            pass
    return inst


def _build():
    nc = bacc.Bacc("TRN2", target_bir_lowering=False, debug=False,
                   enable_asserts=True, num_devices=N_CORES)

    def din(name, shape, dt=F32R):
        return nc.dram_tensor(name, shape, dt, kind="ExternalInput").ap()

    d = {
        "xT": din("xT", [DM, S]),
        "wdqT": din("wdqT", [DM, DL]),
        "wdkvT": din("wdkvT", [DM, DL]),
        "wkrT": din("wkrT", [DM, DHR]),
        "wuqT": din("wuqT", [DL, HC * DH]),
        "wqrT": din("wqrT", [DL, HC * DHR]),
        "wukT": din("wukT", [DL, HC * DH]),
        "wuvT": din("wuvT", [DL, HC * DH]),
        "woT": din("woT", [HC * DH, DM]),
        "ones128": din("ones128", [128, 1]),
        "ones1": din("ones1", [1, 128]),
        "masktri": din("masktri", [128, 128], F32),
        "cs1": din("cs1", [DHR, S], F32),
        "cs2": din("cs2", [DHR, S], F32),
        "outT": nc.dram_tensor("outT", [DM, S], F32,
                               kind="ExternalOutput").ap(),
    }
    with tile.TileContext(nc) as tc:
        import contextlib
        with contextlib.ExitStack() as ctx:
            _kernel_body(ctx, tc, nc, d)
    nc.compile()
    return nc


def _kernel_body(ctx, tc, nc, d):
    wts = ctx.enter_context(tc.tile_pool(name="wts", bufs=1))
    kvp = ctx.enter_context(tc.tile_pool(name="kvp", bufs=1))
    xtp = ctx.enter_context(tc.tile_pool(name="xtp", bufs=1))
    lat = ctx.enter_context(tc.tile_pool(name="lat", bufs=1))
    prj = ctx.enter_context(tc.tile_pool(name="prj", bufs=1))
    smp = ctx.enter_context(tc.tile_pool(name="smp", bufs=1))
    o5p = ctx.enter_context(tc.tile_pool(name="o5p", bufs=3))
    str_p = ctx.enter_context(tc.tile_pool(name="str_p", bufs=5))
    # PSUM: stage-1 dedicated (3) + work rotation (2) + attn (2) + sums (1)
    ps_s1 = ctx.enter_context(tc.tile_pool(name="ps_s1", bufs=1, space="PSUM"))
    ps_w = ctx.enter_context(tc.tile_pool(name="ps_w", bufs=1, space="PSUM"))
    ps_at = ctx.enter_context(tc.tile_pool(name="ps_at", bufs=2, space="PSUM"))
    ps_sm = ctx.enter_context(tc.tile_pool(name="ps_sm", bufs=2, space="PSUM"))

    s1rot = [0]

    def s1tile(shape, name):
        t = ps_s1.tile(shape, F32, tag=f"s1{s1rot[0] % 3}", name=name)
        s1rot[0] += 1
        return t

    # ---- small persistent loads ----
    o128_t = wts.tile([128, 1], F32R, name="o128")
    o1_t = wts.tile([1, 128], F32R, name="o1")
    nc.sync.dma_start(o128_t[:], d["ones128"][:, :])
    nc.sync.dma_start(o1_t[:], d["ones1"][:, :])
    mask_t = wts.tile([128, 128], F32, name="masktri")
    nc.sync.dma_start(mask_t[:], d["masktri"][:, :])

    wkr_t = [wts.tile([128, DHR], F32R, name=f"wkr{m}") for m in range(NMC)]
    for m in range(NMC):
        nc.gpsimd.dma_start(wkr_t[m][:], d["wkrT"][m * 128:(m + 1) * 128, :])
    wdkv_t = [wts.tile([128, DL], F32R, name=f"wdkv{m}") for m in range(NMC)]
    wuq_t = [wts.tile([128, HC * DH], F32R, name=f"wuq{l}") for l in range(NLC)]
    wqr_t = [wts.tile([128, HC * DHR], F32R, name=f"wqr{l}") for l in range(NLC)]
    wuk_t = [wts.tile([128, HC * DH], F32R, name=f"wuk{l}") for l in range(NLC)]
    wuv_t = [wts.tile([128, HC * DH], F32R, name=f"wuv{l}") for l in range(NLC)]
    wo_t = [wts.tile([128, DM], F32R, name=f"wo{h}") for h in range(HC)]

    def emit_wdkv_dmas():
        for m in range(NMC):
            nc.gpsimd.dma_start(wdkv_t[m][:], d["wdkvT"][m * 128:(m + 1) * 128, :])

    def emit_proj_dmas():
        for l in range(NLC):
            nc.gpsimd.dma_start(wuk_t[l][:], d["wukT"][l * 128:(l + 1) * 128, :])
            nc.gpsimd.dma_start(wuv_t[l][:], d["wuvT"][l * 128:(l + 1) * 128, :])
            nc.gpsimd.dma_start(wuq_t[l][:], d["wuqT"][l * 128:(l + 1) * 128, :])
            nc.gpsimd.dma_start(wqr_t[l][:], d["wqrT"][l * 128:(l + 1) * 128, :])

    def emit_wo_dmas():
        for h in range(HC):
            nc.gpsimd.dma_start(wo_t[h][:], d["woT"][h * 128:(h + 1) * 128, :])

    # ---- persistent per-sequence state ----
    kct = [kvp.tile([128, S], F32R, name=f"kct{h}") for h in range(HC)]
    krt = kvp.tile([DHR, S], F32R, name="krt")
    vt = [kvp.tile([128, HC * DH], F32R, name=f"vt{k}") for k in range(NKC)]

    def rope(raw_ps, out_ap, cs1s, cs2s, tag):
        """raw_ps: PSUM [64, QB] pre-rope; out_ap: f32r dest [64, QB]."""
        raw = smp.tile([DHR, QB], F32, tag="rope_raw", name=f"rr_{tag}")
        nc.vector.tensor_copy(raw[:], raw_ps[:])
        rsw = smp.tile([DHR, QB], F32, tag="rope_swp", name=f"rs_{tag}")
        nc.sync.dma_start(rsw[0:32, :], raw[32:64, :])
        nc.sync.dma_start(rsw[32:64, :], raw[0:32, :])
        nc.vector.tensor_tensor(raw[:], raw[:], cs1s[:], op=OP.mult)
        nc.vector.tensor_tensor(rsw[:], rsw[:], cs2s[:], op=OP.mult)
        nc.vector.tensor_tensor(out_ap, raw[:], rsw[:], op=OP.add)

    def stage1(qb, mid_hook=None):
        """Latents in 3 mc-major passes; c_Q weight stream paced by
        co-scheduled kr/ckv matmuls (~200 GB/s demand)."""
        qsl = slice(qb * QB, (qb + 1) * QB)
        xt = [xtp.tile([128, QB], F32R, tag=f"xt{m}", name=f"xt{m}_{qb}")
              for m in range(NMC)]
        for m in range(NMC):
            nc.sync.dma_start(xt[m][:], d["xT"][m * 128:(m + 1) * 128, qsl])
        cs1s = smp.tile([DHR, QB], F32, tag="cs1s", bufs=1, name=f"cs1s{qb}")
        cs2s = smp.tile([DHR, QB], F32, tag="cs2s", bufs=1, name=f"cs2s{qb}")
        nc.sync.dma_start(cs1s[:], d["cs1"][:, qsl])
        nc.sync.dma_start(cs2s[:], d["cs2"][:, qsl])

        ckv = [lat.tile([128, QB], F32R, tag=f"ckv{l}", name=f"ckv{l}_{qb}")
               for l in range(NLC)]
        cq = [lat.tile([128, QB], F32R, tag=f"cq{l}", name=f"cq{l}_{qb}")
              for l in range(NLC)]
        eng_tgl = [0]

        def copy_out(dst, src):
            (nc.vector.tensor_copy if eng_tgl[0] % 2 == 0
             else nc.scalar.copy)(dst, src)
            eng_tgl[0] += 1

        # plan: (wq columns streamed, [groups]) per pass; kr paces pass 0
        plan = [
            ((0, 256), [("kr", None), ("cq", 0), ("cq", 1)]),
            ((256, 512), [("cq", 2), ("cq", 3)]),
            (None, [("ckv", 0), ("ckv", 1)]),
            (None, [("ckv", 2), ("ckv", 3)]),
        ]
        for pi, (wcols, groups) in enumerate(plan):
            pts = []
            for gi, (kind, idx) in enumerate(groups):
                shape = [DHR, QB] if kind == "kr" else [128, QB]
                pts.append(s1tile(shape, f"p{pi}{gi}_{qb}"))
            for m in range(NMC):
                wq = None
                if wcols is not None:
                    lo, hi = wcols
                    wq = str_p.tile([128, 256], F32R, tag="wdqs",
                                    name=f"wq{pi}_{m}_{qb}")
                    nc.sync.dma_start(wq[:],
                                      d["wdqT"][m * 128:(m + 1) * 128, lo:hi])
                for gi, (kind, idx) in enumerate(groups):
                    if kind == "kr":
                        st_ap, label = wkr_t[m][:], "s1_kr"
                    elif kind == "cq":
                        c0 = idx * 128 - wcols[0]
                        st_ap, label = wq[:, c0:c0 + 128], "s1_cq"
                    else:
                        st_ap = wdkv_t[m][:, idx * 128:(idx + 1) * 128]
                        label = "s1_ckv"
                    _lbl(nc.tensor.matmul(pts[gi][:], st_ap, xt[m][:],
                                          start=(m == 0),
                                          stop=(m == NMC - 1)), label)
            for gi, (kind, idx) in enumerate(groups):
                if kind == "kr":
                    rope(pts[gi], krt[:, qsl], cs1s, cs2s, f"kr{qb}")
                elif kind == "cq":
                    copy_out(cq[idx][:], pts[gi][:])
                else:
                    copy_out(ckv[idx][:], pts[gi][:])
            if pi == 1 and mid_hook is not None:
                mid_hook()
        return cq, ckv

    def stage2(qb, cq, ckv):
        qsl = slice(qb * QB, (qb + 1) * QB)
        cs1s = smp.tile([DHR, QB], F32, tag="cs1s", bufs=1, name=f"cs1sq{qb}")
        cs2s = smp.tile([DHR, QB], F32, tag="cs2s", bufs=1, name=f"cs2sq{qb}")
        nc.sync.dma_start(cs1s[:], d["cs1"][:, qsl])
        nc.sync.dma_start(cs2s[:], d["cs2"][:, qsl])
        # q_C / q_R per head
        qct = [prj.tile([128, QB], F32R, tag=f"qct{h}", name=f"qct{h}_{qb}")
               for h in range(HC)]
        qrt = [prj.tile([DHR, QB], F32R, tag=f"qrt{h}", name=f"qrt{h}_{qb}")
               for h in range(HC)]
        for h in range(HC):
            pqc = s1tile([128, QB], f"pqc{h}_{qb}")
            for l in range(NLC):
                _lbl(nc.tensor.matmul(pqc[:], wuq_t[l][:, h * DH:(h + 1) * DH],
                                 cq[l][:], start=(l == 0), stop=(l == NLC - 1)), "s2_qc")
            nc.vector.tensor_copy(qct[h][:], pqc[:])
            pqr = s1tile([DHR, QB], f"pqr{h}_{qb}")
            for l in range(NLC):
                _lbl(nc.tensor.matmul(pqr[:], wqr_t[l][:, h * DHR:(h + 1) * DHR],
                                 cq[l][:], start=(l == 0), stop=(l == NLC - 1)), "s2_qr")
            rope(pqr, qrt[h][:], cs1s, cs2s, f"qr{h}_{qb}")
        # k_C per head into persistent K cache
        for h in range(HC):
            pkc = s1tile([128, QB], f"pkc{h}_{qb}")
            for l in range(NLC):
                _lbl(nc.tensor.matmul(pkc[:], wuk_t[l][:, h * DH:(h + 1) * DH],
                                 ckv[l][:], start=(l == 0), stop=(l == NLC - 1)), "s2_kc")
            (nc.vector.tensor_copy if h == 0 else nc.scalar.copy)(
                kct[h][:, qsl], pkc[:])
        # V chunks (natural layout, both heads packed)
        for sc in range(KPB):
            k = qb * KPB + sc
            pv = s1tile([128, HC * DH], f"pvv{k}")
            for l in range(NLC):
                _lbl(nc.tensor.matmul(pv[:], ckv[l][:, sc * 128:(sc + 1) * 128],
                                 wuv_t[l][:], start=(l == 0), stop=(l == NLC - 1)), "s2_v")
            (nc.vector.tensor_copy if sc % 2 == 0 else nc.scalar.copy)(
                vt[k][:], pv[:])
        return qct, qrt

    def attn_both(qb, qct, qrt):
        """Both heads interleaved per key chunk: 2x PE density per chain step."""
        nkc = KPB * (qb + 1)
        pat = [ps_at.tile([128, QB], F32, tag="at", name=f"pat{h}_{qb}")
               for h in range(HC)]
        psums = [ps_sm.tile([1, QB], F32, tag="smrb", name=f"psums{h}_{qb}")
                 for h in range(HC)]
        pend = []  # (h, kc, off, pt) awaiting PV+sums

        def flush(last):
            h, kc, off, pt = pend.pop(0)
            _lbl(nc.tensor.matmul(psums[h][:, off:], o128_t[:], pt[:, off:],
                                  start=(kc == 0), stop=last,
                                  skip_group_check=True), "sum")
            _lbl(nc.tensor.matmul(pat[h][:, off:],
                                  vt[kc][:, h * DH:(h + 1) * DH],
                                  pt[:, off:], start=(kc == 0), stop=last,
                                  skip_group_check=True), "pv")

        for kc in range(nkc):
            off = 128 * (kc - KPB * qb) if kc >= KPB * qb else 0
            w = QB - off
            ksl = slice(kc * 128, (kc + 1) * 128)
            for h in range(HC):
                ps_s = s1tile([128, QB], f"s{h}_{qb}_{kc}")
                _lbl(nc.tensor.matmul(ps_s[:, off:], kct[h][:, ksl],
                                      qct[h][:, off:], start=True, stop=False,
                                      skip_group_check=True), "qk_c")
                _lbl(nc.tensor.matmul(ps_s[:, off:], krt[:, ksl],
                                      qrt[h][:, off:], start=False, stop=True,
                                      skip_group_check=True), "qk_r")
                if len(pend) >= 2:
                    flush(False)
                et = smp.tile([128, QB], F32, tag="et", bufs=3,
                              name=f"et{h}_{qb}_{kc}")
                nc.scalar.activation(et[:, off:], ps_s[:, off:], AF.Exp,
                                     scale=SCALE)
                pt = smp.tile([128, QB], F32R, tag="pt", bufs=4,
                              name=f"pt{h}_{qb}_{kc}")
                if kc >= KPB * qb:  # diagonal: clip+mask window, clip rest
                    ctw = smp.tile([128, 128], F32, tag="ctw", bufs=2,
                                   name=f"ctw{h}_{qb}_{kc}")
                    nc.vector.tensor_scalar(ctw[:], et[:, off:off + 128],
                                            E_HI, E_LO, op0=OP.min, op1=OP.max)
                    nc.vector.tensor_tensor(pt[:, off:off + 128], ctw[:],
                                            mask_t[:], op=OP.mult)
                    if w > 128:
                        nc.vector.tensor_scalar(pt[:, off + 128:],
                                                et[:, off + 128:], E_HI, E_LO,
                                                op0=OP.min, op1=OP.max)
                else:
                    nc.vector.tensor_scalar(pt[:], et[:], E_HI, E_LO,
                                            op0=OP.min, op1=OP.max)
                pend.append((h, kc, off, pt))
        while len(pend) > 2:
            flush(False)
        while pend:
            flush(True)
        return pat, psums

    def attn_sum(qb, h, psums):  # psums: [1,QB] tile
        rc = smp.tile([1, QB], F32, tag="rc", bufs=1, name=f"rc{h}_{qb}")
        nc.vector.reciprocal(rc[:], psums[:])
        rcr = smp.tile([1, QB], F32R, tag="rcr", bufs=1, name=f"rcr{h}_{qb}")
        nc.vector.tensor_copy(rcr[:], rc[:])
        return rcr

    def attn_norm(qb, h, pat, rcr, attn_n):
        prb = ps_w.tile([128, QB], F32, tag="w", name=f"prb{h}_{qb}")
        _lbl(nc.tensor.matmul(prb[:], o1_t[:], rcr[:], start=True, stop=True), "bcast")
        rbs = smp.tile([128, QB], F32, tag="rbs", bufs=1, name=f"rbs{h}_{qb}")
        nc.scalar.copy(rbs[:], prb[:])
        nc.vector.tensor_tensor(attn_n[:], pat[:], rbs[:], op=OP.mult)

    def stage5(qb, attn_n):
        qsl = slice(qb * QB, (qb + 1) * QB)
        for m in range(NMC):
            po = s1tile([128, QB], f"po{m}_{qb}")
            for h in range(HC):
                _lbl(nc.tensor.matmul(po[:], wo_t[h][:, m * 128:(m + 1) * 128],
                                 attn_n[h][:], start=(h == 0),
                                 stop=(h == HC - 1)), "s5")
            ob = o5p.tile([128, QB], F32, tag="ob", name=f"ob{m}_{qb}")
            (nc.vector.tensor_copy if m % 2 == 0 else nc.scalar.copy)(
                ob[:], po[:])
            nc.sync.dma_start(d["outT"][m * 128:(m + 1) * 128, qsl], ob[:])

    # ---- software-pipelined main loop ----
    def first_hooks():
        emit_wdkv_dmas()

    cq, ckv = stage1(0, mid_hook=first_hooks)
    emit_proj_dmas()
    emit_wo_dmas()
    for qb in range(NQB):
        qct, qrt = stage2(qb, cq, ckv)
        attn_n = [prj.tile([128, QB], F32R, tag=f"an{h}", name=f"an{h}_{qb}")
                  for h in range(HC)]
        pat, psums = attn_both(qb, qct, qrt)
        rcr0 = attn_sum(qb, 0, psums[0])
        rcr1 = attn_sum(qb, 1, psums[1])
        if qb < NQB - 1:
            cq, ckv = stage1(qb + 1)  # hides the reciprocal chains
        attn_norm(qb, 0, pat[0], rcr0, attn_n[0][:])
        attn_norm(qb, 1, pat[1], rcr1, attn_n[1][:])
        stage5(qb, attn_n)


def _prep_inputs(x, W_DQ, W_UQ, W_QR, W_DKV, W_UK, W_UV, W_KR, W_O):
    """Host-side sharding + layout prep. Returns list of 8 in_maps."""
    f32 = np.float32
    xT = np.ascontiguousarray(x[0].T).astype(f32, copy=False)
    perm = np.concatenate([np.arange(0, DHR, 2), np.arange(1, DHR, 2)])
    wdqT = np.ascontiguousarray(W_DQ.T).astype(f32, copy=False)
    wdkvT = np.ascontiguousarray(W_DKV.T).astype(f32, copy=False)
    wkrT = np.ascontiguousarray(W_KR.T[:, perm]).astype(f32, copy=False)

    # rope tables (transposed, permuted-channel layout)
    pos = np.arange(S, dtype=np.float64)
    inv = THETA ** (-np.arange(0, DHR, 2, dtype=np.float64) / DHR)  # (32,)
    ang = inv[:, None] * pos[None, :]                               # (32, S)
    cosv = np.cos(ang).astype(f32)
    sinv = np.sin(ang).astype(f32)
    cs1 = np.ascontiguousarray(np.concatenate([cosv, cosv], axis=0))
    cs2 = np.ascontiguousarray(np.concatenate([-sinv, sinv], axis=0))

    # triangle mask for the 128-wide diagonal window: allow k <= q
    kk = np.arange(128)[:, None]
    qq = np.arange(128)[None, :]
    masktri = np.ascontiguousarray((kk <= qq).astype(f32))

    shared = {
        "xT": xT, "wdqT": wdqT, "wdkvT": wdkvT, "wkrT": wkrT,
        "masktri": masktri, "cs1": cs1, "cs2": cs2,
        "ones128": np.ones((128, 1), f32), "ones1": np.ones((1, 128), f32),
    }
    in_maps = []
    for c in range(N_CORES):
        hs = [c * HC + h for h in range(HC)]
        wuqT = np.concatenate(
            [W_UQ[h * DH:(h + 1) * DH, :].T for h in hs], axis=1)
        wqrT = np.concatenate(
            [W_QR[h * DHR:(h + 1) * DHR, :].T[:, perm] for h in hs], axis=1)
        wukT = np.concatenate(
            [W_UK[h * DH:(h + 1) * DH, :].T for h in hs], axis=1)
        wuvT = np.concatenate(
            [W_UV[h * DH:(h + 1) * DH, :].T for h in hs], axis=1)
        woT = np.concatenate(
            [W_O[:, h * DH:(h + 1) * DH].T for h in hs], axis=0)
        in_maps.append({
            **shared,
            "wuqT": np.ascontiguousarray(wuqT).astype(f32, copy=False),
            "wqrT": np.ascontiguousarray(wqrT).astype(f32, copy=False),
            "wukT": np.ascontiguousarray(wukT).astype(f32, copy=False),
            "wuvT": np.ascontiguousarray(wuvT).astype(f32, copy=False),
            "woT": np.ascontiguousarray(woT).astype(f32, copy=False),
        })
    return in_maps


def kernel(**inputs):
    global LAST_EXEC_TIME_NS, LAST_RESULTS
    if "nc" not in _CACHE:
        _CACHE["nc"] = _build()
    nc = _CACHE["nc"]
    in_maps = _prep_inputs(**{k: np.asarray(v) for k, v in inputs.items()})
    kwargs = dict(TRACE_KWARGS)
    if TRACE:
        kwargs["trace"] = True
    res = run_bass_kernel_spmd(nc, in_maps, core_ids=list(range(N_CORES)),
                               **kwargs)
    LAST_EXEC_TIME_NS = res.exec_time_ns
    LAST_RESULTS = res
    acc = np.zeros((DM, S), np.float64)
    for c in range(N_CORES):
        acc += res.results[c]["outT"].astype(np.float64)
    return np.ascontiguousarray(acc.T[None]).astype(np.float32)



# revision 13
# speedup vs baseline: 1.6071x; 1.6071x over previous
"""Multi-Head Latent Attention (MLA) Trainium2 kernel, 8-core head-sharded.

v2: the latent down-projections are folded into the per-head up-projections
on the host (W_xQ = (W_UQ_h W_DQ)^T etc.), so the device never computes the
replicated 512-wide latents — each core runs only its 2 heads' fused
projections straight from x. All matmul operands are bf16 (same PE rate as
f32r, half the SBUF/DMA), accumulation in f32 PSUM. Softmax denominators
accumulate on the Vector engine (no per-chunk ones-matmul), collapsed once
per block with a single matmul, reciprocal via the fast DVE approximation.

Pipeline per query block qb: attn(qb) -> [sum collapse] -> proj(qb+1) fills
the PE while the softmax tail (reciprocal, broadcast, normalize) runs on
DVE/ACT -> out-proj(qb).
"""
import sys

sys.path.insert(0, "/opt/trn_rl_repo")

import numpy as np
import ml_dtypes

import concourse.bass as bass
import concourse.tile as tile
from concourse import bacc, mybir
from concourse.bass_utils import run_bass_kernel_spmd

F32 = mybir.dt.float32
F32R = mybir.dt.float32r
BF16 = mybir.dt.bfloat16
AF = mybir.ActivationFunctionType
OP = mybir.AluOpType

N_CORES = 8
S = 2048          # sequence length
DM = 2048         # d_model
H = 16            # total heads
HC = H // N_CORES  # heads per core (2)
DH = 128          # head dim (content)
DHR = 64          # head dim (rope)
QB = 512          # query block
NQB = S // QB     # 4
NMC = DM // 128   # 16 model chunks
NKC = S // 128    # 16 key chunks
THETA = 10000.0

SCALE = float(1.0 / np.sqrt(np.float32(DH + DHR)))
E_HI = float(np.exp(np.float64(80.0) * SCALE))
E_LO = float(np.exp(np.float64(-80.0) * SCALE))

# Set by test.py to profile; harness path leaves these untouched.
TRACE = False
TRACE_KWARGS = {}
LAST_EXEC_TIME_NS = None
LAST_RESULTS = None

_CACHE = {}


def _build():
    nc = bacc.Bacc("TRN2", target_bir_lowering=False, debug=False,
                   enable_asserts=True, num_devices=N_CORES)

    def din(name, shape, dt=BF16):
        return nc.dram_tensor(name, shape, dt, kind="ExternalInput").ap()

    d = {
        "xT": din("xT", [DM, S]),
        "wq": din("wq", [DM, HC * DH]),
        "wk": din("wk", [DM, HC * DH]),
        "wv": din("wv", [DM, HC * DH]),
        "wqrkr": din("wqrkr", [DM, (HC + 1) * DHR]),
        "woT": din("woT", [HC * DH, DM]),
        "ones128": din("ones128", [128, 1], F32R),
        "ones1": din("ones1", [1, 128], F32R),
        "masktri": din("masktri", [128, 128], F32),
        "cs1": din("cs1", [128, S], F32),
        "cs2": din("cs2", [128, S], F32),
        "outT": nc.dram_tensor("outT", [DM, S], F32,
                               kind="ExternalOutput").ap(),
    }
    with tile.TileContext(nc) as tc:
        import contextlib
        with contextlib.ExitStack() as ctx:
            _kernel_body(ctx, tc, nc, d)
    nc.compile()
    return nc


def _kernel_body(ctx, tc, nc, d):
    wts = ctx.enter_context(tc.tile_pool(name="wts", bufs=1))
    kvp = ctx.enter_context(tc.tile_pool(name="kvp", bufs=1))
    xtp = ctx.enter_context(tc.tile_pool(name="xtp", bufs=2))
    prj = ctx.enter_context(tc.tile_pool(name="prj", bufs=2))
    smp = ctx.enter_context(tc.tile_pool(name="smp", bufs=1))
    o5p = ctx.enter_context(tc.tile_pool(name="o5p", bufs=3))
    ps_p = ctx.enter_context(tc.tile_pool(name="ps_p", bufs=3, space="PSUM"))
    ps_s = ctx.enter_context(tc.tile_pool(name="ps_s", bufs=2, space="PSUM"))
    ps_at = ctx.enter_context(tc.tile_pool(name="ps_at", bufs=2, space="PSUM"))

    # ---- small persistent loads ----
    o128_t = wts.tile([128, 1], F32R, name="o128")
    o1_t = wts.tile([1, 128], F32R, name="o1")
    nc.sync.dma_start(o128_t[:], d["ones128"][:, :])
    nc.sync.dma_start(o1_t[:], d["ones1"][:, :])
    mask_t = wts.tile([128, 128], F32, name="masktri")
    nc.sync.dma_start(mask_t[:], d["masktri"][:, :])
    cs1_t = wts.tile([128, S], F32, name="cs1")
    cs2_t = wts.tile([128, S], F32, name="cs2")
    nc.gpsimd.dma_start(cs1_t[:], d["cs1"][:, :])
    nc.gpsimd.dma_start(cs2_t[:], d["cs2"][:, :])

    # ---- persistent weights (bf16) ----
    wq_t = [wts.tile([128, HC * DH], BF16, name=f"wq{m}") for m in range(NMC)]
    wk_t = [wts.tile([128, HC * DH], BF16, name=f"wk{m}") for m in range(NMC)]
    wv_t = [wts.tile([128, HC * DH], BF16, name=f"wv{m}") for m in range(NMC)]
    wr_t = [wts.tile([128, (HC + 1) * DHR], BF16, name=f"wr{m}")
            for m in range(NMC)]
    wo_t = [wts.tile([128, DM], BF16, name=f"wo{h}") for h in range(HC)]
    for m in range(NMC):
        nc.gpsimd.dma_start(wq_t[m][:], d["wq"][m * 128:(m + 1) * 128, :])
    for m in range(NMC):
        nc.gpsimd.dma_start(wk_t[m][:], d["wk"][m * 128:(m + 1) * 128, :])
    for m in range(NMC):
        nc.gpsimd.dma_start(wr_t[m][:], d["wqrkr"][m * 128:(m + 1) * 128, :])
    for m in range(NMC):
        nc.gpsimd.dma_start(wv_t[m][:], d["wv"][m * 128:(m + 1) * 128, :])
    for h in range(HC):
        nc.gpsimd.dma_start(wo_t[h][:], d["woT"][h * 128:(h + 1) * 128, :])

    # ---- persistent per-sequence state (bf16) ----
    kct = [kvp.tile([128, S], BF16, name=f"kct{h}") for h in range(HC)]
    krt2 = kvp.tile([128, S], BF16, name="krt2")  # rope-k duplicated 2x64
    vt = [kvp.tile([128, HC * DH], BF16, name=f"vt{k}") for k in range(NKC)]

    def rope_packed(raw_ps, out_ap, qsl, tag):
        """raw_ps: PSUM [128, QB] pre-rope (two 64-blocks); out: bf16."""
        raw = smp.tile([128, QB], F32, tag="rope_raw", bufs=2,
                       name=f"rr_{tag}")
        nc.vector.tensor_copy(raw[:], raw_ps[:])
        rsw = smp.tile([128, QB], F32, tag="rope_swp", bufs=2,
                       name=f"rs_{tag}")
        for b in range(0, 128, 64):
            nc.sync.dma_start(rsw[b:b + 32, :], raw[b + 32:b + 64, :])
            nc.sync.dma_start(rsw[b + 32:b + 64, :], raw[b:b + 32, :])
        nc.vector.tensor_tensor(raw[:], raw[:], cs1_t[:, qsl], op=OP.mult)
        nc.vector.tensor_tensor(rsw[:], rsw[:], cs2_t[:, qsl], op=OP.mult)
        nc.vector.tensor_tensor(out_ap, raw[:], rsw[:], op=OP.add)

    def proj(qb, mid_hook=None):
        """Fused projections for block qb: q/k/v/qr/kr straight from x."""
        qsl = slice(qb * QB, (qb + 1) * QB)
        xt = [xtp.tile([128, QB], BF16, tag=f"xt{m}", name=f"xt{m}_{qb}")
              for m in range(NMC)]
        for m in range(NMC):
            nc.sync.dma_start(xt[m][:], d["xT"][m * 128:(m + 1) * 128, qsl])
        qct = [prj.tile([128, QB], BF16, tag=f"qct{h}", name=f"qct{h}_{qb}")
               for h in range(HC)]
        qrt = prj.tile([128, QB], BF16, tag="qrt", name=f"qrt_{qb}")
        eng = [0]

        def copy_out(dst, src):
            (nc.vector.tensor_copy if eng[0] % 2 == 0
             else nc.scalar.copy)(dst, src)
            eng[0] += 1

        # q/k passes (one [128,QB] accumulator each)
        for h in range(HC):
            pq = ps_p.tile([128, QB], F32, tag="pp", name=f"pq{h}_{qb}")
            for m in range(NMC):
                nc.tensor.matmul(pq[:], wq_t[m][:, h * DH:(h + 1) * DH],
                                 xt[m][:], start=(m == 0), stop=(m == NMC - 1))
            copy_out(qct[h][:], pq[:])
        for h in range(HC):
            pk = ps_p.tile([128, QB], F32, tag="pp", name=f"pk{h}_{qb}")
            for m in range(NMC):
                nc.tensor.matmul(pk[:], wk_t[m][:, h * DH:(h + 1) * DH],
                                 xt[m][:], start=(m == 0), stop=(m == NMC - 1))
            copy_out(kct[h][:, qsl], pk[:])
        if mid_hook is not None:
            mid_hook()
        # qr pass (both heads packed [128, QB])
        pqr = ps_p.tile([128, QB], F32, tag="pp", name=f"pqr_{qb}")
        for m in range(NMC):
            nc.tensor.matmul(pqr[:], wr_t[m][:, 0:128], xt[m][:],
                             start=(m == 0), stop=(m == NMC - 1))
        rope_packed(pqr, qrt[:], qsl, f"qr{qb}")
        # kr pass ([64, QB] -> duplicated to [128, QB])
        pkrt = ps_p.tile([128, QB], F32, tag="pp", name=f"pkr_{qb}")
        pkr = pkrt[0:64, :]
        for m in range(NMC):
            nc.tensor.matmul(pkr, wr_t[m][:, 128:192], xt[m][:],
                             start=(m == 0), stop=(m == NMC - 1))
        krd = smp.tile([128, QB], F32, tag="krd", bufs=2, name=f"krd_{qb}")
        nc.vector.tensor_copy(krd[0:64, :], pkr)
        nc.scalar.copy(krd[64:128, :], pkr)
        pkr2 = smp.tile([128, QB], F32, tag="krd2", bufs=2, name=f"krd2_{qb}")
        for b in range(0, 128, 64):
            nc.sync.dma_start(pkr2[b:b + 32, :], krd[b + 32:b + 64, :])
            nc.sync.dma_start(pkr2[b + 32:b + 64, :], krd[b:b + 32, :])
        nc.vector.tensor_tensor(krd[:], krd[:], cs1_t[:, qsl], op=OP.mult)
        nc.vector.tensor_tensor(pkr2[:], pkr2[:], cs2_t[:, qsl], op=OP.mult)
        nc.vector.tensor_tensor(krt2[:, qsl], krd[:], pkr2[:], op=OP.add)
        # v pass (natural [keys, HC*DH] layout)
        for sc in range(QB // 128):
            pv = ps_p.tile([128, HC * DH], F32, tag="pp", name=f"pv{sc}_{qb}")
            for m in range(NMC):
                nc.tensor.matmul(pv[:], xt[m][:, sc * 128:(sc + 1) * 128],
                                 wv_t[m][:], start=(m == 0),
                                 stop=(m == NMC - 1))
            copy_out(vt[qb * (QB // 128) + sc][:], pv[:])
        return qct, qrt

    def attn(qb, qct, qrt):
        """Both heads interleaved per key chunk; DVE-accumulated sums."""
        nkc = (QB // 128) * (qb + 1)
        pat = [ps_at.tile([128, QB], F32, tag="at", name=f"pat{h}_{qb}")
               for h in range(HC)]
        sumacc = [smp.tile([128, QB], F32R, tag=f"sum{h}", bufs=2,
                           name=f"sum{h}_{qb}") for h in range(HC)]
        pend = []  # (h, kc, off, pt) awaiting PV

        def flush(last):
            h, kc, off, pt = pend.pop(0)
            nc.tensor.matmul(pat[h][:, off:],
                             vt[kc][:, h * DH:(h + 1) * DH],
                             pt[:, off:], start=(kc == 0), stop=last,
                             skip_group_check=True)

        for kc in range(nkc):
            off = 128 * (kc - (QB // 128) * qb) if kc >= (QB // 128) * qb else 0
            w = QB - off
            ksl = slice(kc * 128, (kc + 1) * 128)
            for h in range(HC):
                pss = ps_s.tile([128, QB], F32, tag="ss", name=f"s{h}_{qb}_{kc}")
                nc.tensor.matmul(pss[:, off:], kct[h][:, ksl],
                                 qct[h][:, off:], start=True, stop=False,
                                 skip_group_check=True)
                nc.tensor.matmul(pss[:, off:],
                                 krt2[h * DHR:(h + 1) * DHR, ksl],
                                 qrt[h * DHR:(h + 1) * DHR, off:],
                                 start=False, stop=True,
                                 skip_group_check=True)
                if len(pend) >= 2:
                    flush(False)
                et = smp.tile([128, QB], F32, tag="et", bufs=3,
                              name=f"et{h}_{qb}_{kc}")
                nc.scalar.activation(et[:, off:], pss[:, off:], AF.Exp,
                                     scale=SCALE)
                pt = smp.tile([128, QB], BF16, tag="pt", bufs=4,
                              name=f"pt{h}_{qb}_{kc}")
                if kc >= (QB // 128) * qb:  # diagonal: clip+mask window
                    ctw = smp.tile([128, 128], F32, tag="ctw", bufs=2,
                                   name=f"ctw{h}_{qb}_{kc}")
                    nc.vector.tensor_scalar(ctw[:], et[:, off:off + 128],
                                            E_HI, E_LO, op0=OP.min, op1=OP.max)
                    nc.vector.tensor_tensor(pt[:, off:off + 128], ctw[:],
                                            mask_t[:], op=OP.mult)
                    if w > 128:
                        nc.vector.tensor_scalar(pt[:, off + 128:],
                                                et[:, off + 128:], E_HI, E_LO,
                                                op0=OP.min, op1=OP.max)
                else:
                    nc.vector.tensor_scalar(pt[:], et[:], E_HI, E_LO,
                                            op0=OP.min, op1=OP.max)
                if kc == 0:
                    nc.vector.tensor_copy(sumacc[h][:], pt[:])
                else:
                    nc.vector.tensor_tensor(sumacc[h][:, off:],
                                            sumacc[h][:, off:], pt[:, off:],
                                            op=OP.add)
                pend.append((h, kc, off, pt))
        while len(pend) > 2:
            flush(False)
        while pend:
            flush(True)
        # collapse sums: [128, QB] -> [1, QB] per head
        psums = []
        for h in range(HC):
            pt_s = ps_p.tile([128, QB], F32, tag="pp", name=f"psum{h}_{qb}")
            nc.tensor.matmul(pt_s[0:1, :], o128_t[:], sumacc[h][:],
                             start=True, stop=True)
            psums.append(pt_s)
        return pat, psums

    def tail_recip(qb, h, psum):
        """DVE part of the softmax tail: reciprocal of the denominators."""
        rc = smp.tile([1, QB], F32, tag=f"rc{h}", bufs=2, name=f"rc{h}_{qb}")
        nc.vector.reciprocal_approx_fast(rc[:], psum[0:1, :])
        rcr = smp.tile([1, QB], F32R, tag=f"rcr{h}", bufs=2,
                       name=f"rcr{h}_{qb}")
        nc.vector.tensor_copy(rcr[:], rc[:])
        return rcr

    def tail_norm(qb, h, pat, rcr, attn_n):
        """Broadcast 1/sum along partitions (tiny matmul) and normalize."""
        prb = ps_s.tile([128, QB], F32, tag="ss", name=f"prb{h}_{qb}")
        nc.tensor.matmul(prb[:], o1_t[:], rcr[:], start=True, stop=True)
        rbs = smp.tile([128, QB], F32, tag=f"rbs{h}", bufs=2,
                       name=f"rbs{h}_{qb}")
        nc.scalar.copy(rbs[:], prb[:])
        nc.vector.tensor_tensor(attn_n[:], pat[:], rbs[:], op=OP.mult)

    def outproj(qb, attn_n):
        qsl = slice(qb * QB, (qb + 1) * QB)
        for m in range(NMC):
            po = ps_p.tile([128, QB], F32, tag="pp", name=f"po{m}_{qb}")
            for h in range(HC):
                nc.tensor.matmul(po[:], wo_t[h][:, m * 128:(m + 1) * 128],
                                 attn_n[h][:], start=(h == 0),
                                 stop=(h == HC - 1))
            ob = o5p.tile([128, QB], F32, tag="ob", name=f"ob{m}_{qb}")
            (nc.vector.tensor_copy if m % 2 == 0 else nc.scalar.copy)(
                ob[:], po[:])
            nc.sync.dma_start(d["outT"][m * 128:(m + 1) * 128, qsl], ob[:])

    # ---- software-pipelined main loop ----
    qct, qrt = proj(0)
    for qb in range(NQB):
        pat, psums = attn(qb, qct, qrt)
        attn_n = [prj.tile([128, QB], BF16, tag=f"an{h}", name=f"an{h}_{qb}")
                  for h in range(HC)]
        rcrs = [tail_recip(qb, h, psums[h]) for h in range(HC)]

        def mid(qb=qb, pat=pat, rcrs=rcrs, attn_n=attn_n):
            for h in range(HC):
                tail_norm(qb, h, pat[h], rcrs[h], attn_n[h][:])

        if qb < NQB - 1:
            qct, qrt = proj(qb + 1, mid_hook=mid)  # hides the softmax tail
        else:
            mid()
        outproj(qb, attn_n)


def _prep_inputs(x, W_DQ, W_UQ, W_QR, W_DKV, W_UK, W_UV, W_KR, W_O):
    """Host-side weight fusion + sharding + layout prep -> 8 in_maps."""
    f32 = np.float32
    bf16 = ml_dtypes.bfloat16
    xT = np.ascontiguousarray(x[0].astype(f32).T).astype(bf16)
    perm = np.concatenate([np.arange(0, DHR, 2), np.arange(1, DHR, 2)])

    # fused projection matrices (f32 on host)
    Aq = (W_UQ.astype(f32) @ W_DQ.astype(f32))      # [H*DH, DM]
    Ak = (W_UK.astype(f32) @ W_DKV.astype(f32))
    Av = (W_UV.astype(f32) @ W_DKV.astype(f32))
    Aqr = (W_QR.astype(f32) @ W_DQ.astype(f32))     # [H*DHR, DM]

    # rope tables (transposed, permuted-channel layout, 2x64 blocks)
    pos = np.arange(S, dtype=np.float64)
    inv = THETA ** (-np.arange(0, DHR, 2, dtype=np.float64) / DHR)  # (32,)
    ang = inv[:, None] * pos[None, :]                               # (32, S)
    cosv = np.cos(ang).astype(f32)
    sinv = np.sin(ang).astype(f32)
    blk1 = np.concatenate([cosv, cosv], axis=0)      # (64, S)
    blk2 = np.concatenate([-sinv, sinv], axis=0)
    cs1 = np.ascontiguousarray(np.concatenate([blk1, blk1], axis=0))
    cs2 = np.ascontiguousarray(np.concatenate([blk2, blk2], axis=0))

    kk = np.arange(128)[:, None]
    qq = np.arange(128)[None, :]
    masktri = np.ascontiguousarray((kk <= qq).astype(f32))

    wkrT = W_KR.astype(f32).T[:, perm]               # [DM, 64]

    shared = {
        "xT": xT, "masktri": masktri, "cs1": cs1, "cs2": cs2,
        "ones128": np.ones((128, 1), f32), "ones1": np.ones((1, 128), f32),
    }
    in_maps = []
    for c in range(N_CORES):
        hs = [c * HC + h for h in range(HC)]
        wq = np.concatenate(
            [Aq[h * DH:(h + 1) * DH, :].T for h in hs], axis=1)
        wk = np.concatenate(
            [Ak[h * DH:(h + 1) * DH, :].T for h in hs], axis=1)
        wv = np.concatenate(
            [Av[h * DH:(h + 1) * DH, :].T for h in hs], axis=1)
        wqrkr = np.concatenate(
            [Aqr[h * DHR:(h + 1) * DHR, :].T[:, perm] for h in hs]
            + [wkrT], axis=1)                         # [DM, 192]
        woT = np.concatenate(
            [W_O[:, h * DH:(h + 1) * DH].astype(f32).T for h in hs], axis=0)
        in_maps.append({
            **shared,
            "wq": np.ascontiguousarray(wq).astype(bf16),
            "wk": np.ascontiguousarray(wk).astype(bf16),
            "wv": np.ascontiguousarray(wv).astype(bf16),
            "wqrkr": np.ascontiguousarray(wqrkr).astype(bf16),
            "woT": np.ascontiguousarray(woT).astype(bf16),
        })
    return in_maps


def kernel(**inputs):
    global LAST_EXEC_TIME_NS, LAST_RESULTS
    if "nc" not in _CACHE:
        _CACHE["nc"] = _build()
    nc = _CACHE["nc"]
    in_maps = _prep_inputs(**{k: np.asarray(v) for k, v in inputs.items()})
    kwargs = dict(TRACE_KWARGS)
    if TRACE:
        kwargs["trace"] = True
    res = run_bass_kernel_spmd(nc, in_maps, core_ids=list(range(N_CORES)),
                               **kwargs)
    LAST_EXEC_TIME_NS = res.exec_time_ns
    LAST_RESULTS = res
    acc = np.zeros((DM, S), np.float64)
    for c in range(N_CORES):
        acc += res.results[c]["outT"].astype(np.float64)
    return np.ascontiguousarray(acc.T[None]).astype(np.float32)
